# revision 28
# baseline (speedup 1.0000x reference)
"""Trainium2 Bass kernel for nn_MultiHeadMuonLoRALinear.

Math: out = x @ W^T + bias + sum_h alpha_h * x @ M_h^T, where
M_h = newtonschulz5(B_h @ A_h) and G_h = B_h @ A_h has rank hr=4.

Key algebraic identity: with G = B A (rank hr), every Newton-Schulz
iterate stays in the same row/column space, so X_k = B C_k A for an
hr x hr matrix C_k:
    C_0 = I / (||G||_F + eps),  ||G||_F^2 = tr((B^T B)(A A^T))
    C'  = a C + b (C P C^T) Q C + c (C P C^T Q)^2 C,  P = A A^T, Q = B^T B
Therefore M_h = B_h C_h A_h and the whole LoRA branch collapses to a
rank-16 update:  delta = sum_h alpha_h B_h C_h A_h,  out = x @ (W + delta)^T + bias.

The device kernel computes the single large GEMM (data-parallel over
tokens across 8 cores) with the rank-16 delta folded into W on the
host (0.2% of total FLOPs).

Device strategy (per core, T=1024 tokens, K=O=4096):
  - All-bf16 GEMM, f32 PSUM. The PE streams 1 moving column/cycle for
    every dtype >= bf16 (fp8 DoubleRow is the only 2x mode but fails
    the 2e-2 accuracy budget), so the per-core compute floor is
    32*32*2 matmuls x 512 columns = 437us. bf16 halves DMA vs fp32r
    and, critically, enables FWL (fast weight load), which fp32r's
    FP32-HIGH mode disables — LDWEIGHTS becomes fully hidden and the
    steady matmul issue rate drops from 233ns to 216ns.
  - Phase A k-sweeps the first A=4 o-tiles across the four double-bank
    PSUM groups, consuming x tiles in their DMA arrival order. The x
    tiles and phase-A weight chunks are spread across all three DMA
    queues (sync/scalar HWDGE ~100 GB/s each, gpsimd SWDGE ~210 GB/s)
    in a just-in-time order so the PE never starves while x streams.
  - Phase B runs the remaining o-tiles k-contiguous, tb-outer so each
    half's activation+store overlaps the other half's matmuls; W slabs
    are prefetched on the gpsimd queue, throttled by the pool depth.
  - A short PE warmup (dep-free matmuls) covers the fixed ~9us NEFF
    preamble + first-tile DMA window so the HAM clock gate is released
    before the first real matmul.
"""

import numpy as np
import ml_dtypes

import concourse.bass as bass
import concourse.bacc as bacc
import concourse.mybir as mybir
import concourse.tile as tile
from concourse.bass import ts
from concourse.bass_utils import run_bass_kernel_spmd

N_HEADS = 4
NS_STEPS = 5
NS_EPS = 1e-7
NS_A, NS_B, NS_C = 3.4445, -4.775, 2.0315

N_CORES = 8
P = 128

F32 = mybir.dt.float32
BF16 = mybir.dt.bfloat16

A_OT = 4       # phase-A o-tiles (PSUM: 4 groups x 2 banks = all 8 banks)
KC = 4         # k-tiles per phase-A weight chunk
WARMUP = 11


def host_fold_lora(W, bias, lora_A, lora_B):
    """Collapse the per-head Newton-Schulz into hr x hr space (float64)
    and return W_eff = W + sum_h alpha_h B_h C_h A_h (float32)."""
    r, D_in = lora_A.shape
    D_out = lora_B.shape[0]
    hr = r // N_HEADS
    Ah = lora_A.reshape(N_HEADS, hr, D_in).astype(np.float64)
    Bh = lora_B.reshape(D_out, N_HEADS, hr).transpose(1, 0, 2).astype(np.float64)

    AT = np.zeros((r, D_in))   # rows: alpha-weighted C_h A_h per head
    BT = np.zeros((D_out, r))  # cols: B_h per head
    for h in range(N_HEADS):
        A = Ah[h]
        B = Bh[h]
        Pm = A @ A.T
        Qm = B.T @ B
        fro = np.sqrt(np.trace(Qm @ Pm))
        C = np.eye(hr) / (fro + NS_EPS)
        for _ in range(NS_STEPS):
            D = C @ Pm @ C.T
            E = D @ Qm
            C = NS_A * C + NS_B * (E @ C) + NS_C * (E @ (E @ C))
        AT[h * hr:(h + 1) * hr] = fro * (C @ A)
        BT[:, h * hr:(h + 1) * hr] = B
    delta = BT @ AT
    return (W.astype(np.float64) + delta).astype(np.float32)


def build_bass(K, O, T):
    """Per-core SPMD program: outT[O, T] = (x W_eff^T + bias)^T for this
    core's token shard, all-bf16 GEMM with f32 PSUM accumulation.

    DRAM layouts (host-prepared):
      x:    [128, K//128, T] bf16   x_dev[k, kt, t] = x_shard[t, kt*128 + k]
      w:    [O//128, 128, K//128, 128] bf16  w_dev[ot, k, kt, o] = W_eff[ot*128+o, kt*128+k]
      wa:   [CH, 128, A, KC, 128] bf16  phase-A chunk groups:
            wa_dev[c, k, a, kc, o] = W_eff[a*128+o, (c*KC+kc)*128+k]
      bias: [128, O//128] f32       bias_dev[o, ot] = bias[ot*128 + o]
      out:  [O, T] bf16             outT
    """
    KT, OT = K // P, O // P
    A = A_OT
    CH = KT // KC
    TB = T // 512
    nc = bacc.Bacc()

    x_d = nc.declare_dram_parameter("x", [P, KT, T], BF16, isOutput=False)
    w_d = nc.declare_dram_parameter("w", [OT, P, KT, P], BF16, isOutput=False)
    wa_d = nc.declare_dram_parameter("wa", [CH, P, A, KC, P], BF16,
                                     isOutput=False)
    pkt_d = nc.declare_dram_parameter("pkt", [P, 2 * A * P + 1024], BF16,
                                      isOutput=False)
    b_d = nc.declare_dram_parameter("bias", [P, OT], F32, isOutput=False)
    out_d = nc.declare_dram_parameter("out", [O, T], BF16, isOutput=True)

    with tile.TileContext(nc) as tc:
        with (
            tc.tile_pool(name="xpool", bufs=1) as xpool,
            tc.tile_pool(name="cpool", bufs=1) as cpool,
            tc.tile_pool(name="wapool", bufs=1) as wapool,
            tc.tile_pool(name="wbpool", bufs=6) as wbpool,
            tc.tile_pool(name="opool", bufs=6) as opool,
            tc.tile_pool(name="pspool", bufs=4, space="PSUM") as pspool,
        ):
            # x access: x_tb[kt][tb] -> [128, 512] AP for that k-tile half.
            x_tb = [None] * KT

            def load_x(kt, engine):
                xt = xpool.tile([P, T], BF16, tag=f"x{kt}", name=f"x{kt}")
                engine.dma_start(out=xt[:], in_=x_d[:, kt, :])
                x_tb[kt] = [xt[:, ts(tb, 512)] for tb in range(TB)]

            def load_x_halves(kt, engine):
                # Two half-tiles so the first 512 tokens land ~2us sooner;
                # used for the k-tiles that gate the phase-A start.
                aps = []
                for tb in range(TB):
                    xt = xpool.tile([P, 512], BF16, tag=f"x{kt}_{tb}",
                                    name=f"x{kt}_{tb}")
                    engine.dma_start(out=xt[:],
                                     in_=x_d[:, kt, ts(tb, 512)])
                    aps.append(xt[:])
                x_tb[kt] = aps

            def load_x_batch(k0, k1, engine):
                n = k1 - k0
                xt = xpool.tile([P, n, T], BF16, tag=f"xb{k0}",
                                name=f"xb{k0}")
                engine.dma_start(out=xt[:], in_=x_d[:, k0:k1, :])
                for kt in range(k0, k1):
                    x_tb[kt] = [xt[:, kt - k0, ts(tb, 512)]
                                for tb in range(TB)]

            # Phase-A weights: wlhsT(a, kt) -> [128, 128] stationary AP.
            wa_group = {}    # c -> tile [P, A, KC, P]
            # First packet: kt0+kt1 weights for all A o-tiles plus the
            # first 512-token half of x[kt0], as ONE dma on the fast
            # gpsimd queue so the phase-A start is deterministic (~12us)
            # instead of racing the contended HWDGE queues.
            pkt = [None]

            def load_pkt(engine):
                wt = wapool.tile([P, 2 * A * P + 1024], BF16, tag="pkt",
                                 name="pkt")
                engine.dma_start(out=wt[:], in_=pkt_d[:])
                pkt[0] = wt

            def load_wa_group(c, engine):
                wt = wapool.tile([P, A, KC, P], BF16, tag=f"wag{c}",
                                 name=f"wag{c}")
                engine.dma_start(out=wt[:], in_=wa_d[c])
                wa_group[c] = wt

            def wa_lhsT(a, kt):
                if kt < 2 and pkt[0] is not None:
                    return pkt[0][:, (kt * A + a) * P:(kt * A + a + 1) * P]
                return wa_group[kt // KC][:, a, kt % KC, :]

            wb = {}          # ot -> slab tile [P, KT, P]

            def load_wb(ot, engine):
                wt = wbpool.tile([P, KT, P], BF16, tag="wb", name=f"w{ot}")
                engine.dma_start(out=wt[:], in_=w_d[ot])
                wb[ot] = wt

            bias_sb = cpool.tile([P, OT], F32)

            # ---- queue programs (emission order == per-engine queue order)
            # Just-in-time supply: phase A consumes (x[kt], wa chunk) pairs
            # at ~1.73us per k-tile from t~14us; each item below lands
            # (at ~100 GB/s HWDGE / ~210 GB/s SWDGE) ahead of its deadline.
            # sync HWDGE (~55 GB/s while contended early, ~100 after):
            # x0's first half rides in the gpsimd first packet; load only
            # its second half here.
            x0h1 = xpool.tile([P, 512], BF16, tag="x0_1", name="x0_1")
            nc.sync.dma_start(out=x0h1[:], in_=x_d[:, 0, ts(1, 512)])
            for kt in (2, 4, 6, 12, 14):
                load_x(kt, nc.sync)
            load_x_batch(16, 20, nc.sync)
            load_x_batch(24, 28, nc.sync)
            # scalar HWDGE:
            x1h1 = xpool.tile([P, 512], BF16, tag="x1_1", name="x1_1")
            nc.scalar.dma_start(out=x1h1[:], in_=x_d[:, 1, ts(1, 512)])
            for kt in (3, 5, 7, 13, 15):
                load_x(kt, nc.scalar)
            load_x_batch(20, 24, nc.scalar)
            load_x_batch(28, 32, nc.scalar)
            nc.scalar.dma_start(out=bias_sb[:], in_=b_d[:])
            # gpsimd SWDGE (~190 GB/s, ~1.3us fixed/item): all phase-A
            # weight chunk groups plus the mid x batch, in deadline order.
            load_pkt(nc.gpsimd)
            load_wa_group(0, nc.gpsimd)
            load_wa_group(1, nc.gpsimd)
            load_x_batch(8, 12, nc.gpsimd)
            load_wa_group(2, nc.gpsimd)
            load_wa_group(3, nc.gpsimd)
            load_wa_group(4, nc.gpsimd)
            load_wa_group(5, nc.gpsimd)
            load_wa_group(6, nc.gpsimd)
            load_wa_group(7, nc.gpsimd)
            load_wb(4, nc.gpsimd)
            load_wb(5, nc.gpsimd)
            base = 2 * A * P
            x_tb[0] = [pkt[0][:, base:base + 512], x0h1[:]]
            x_tb[1] = [pkt[0][:, base + 512:base + 1024], x1h1[:]]

            # PE warmup across the preamble + first-tile DMA window.
            wu_src = cpool.tile([P, 512], BF16, name="wu_src")
            nc.vector.memset(wu_src[:], 0.0)
            wu_ps = pspool.tile([P, T], F32, tag="ps", name="wu_ps")
            for i in range(WARMUP):
                nc.tensor.matmul(
                    wu_ps[:, :512], lhsT=wu_src[:, :P], rhs=wu_src[:],
                    start=(i == 0), stop=(i == WARMUP - 1),
                )

            # Phase A: k-outer sweep over the first A o-tiles in parallel
            # PSUM groups. o-tile order 2,3,0,1 at each k so the first
            # matmuls wait on the earliest-arriving weight chunks.
            ps_a = [pspool.tile([P, T], F32, tag="ps", name=f"psA{a}")
                    for a in range(A)]
            for kt in range(KT):
                for a in range(A):
                    for tb in range(TB):
                        nc.tensor.matmul(
                            ps_a[a][:, ts(tb, 512)],
                            lhsT=wa_lhsT(a, kt),
                            rhs=x_tb[kt][tb],
                            start=(kt == 0),
                            stop=(kt == KT - 1),
                        )

            def emit_part(ot, ps, j, width):
                out_sb = opool.tile([P, width], BF16)
                nc.scalar.activation(
                    out_sb[:],
                    ps[:, ts(j, width)],
                    mybir.ActivationFunctionType.Identity,
                    bias=bias_sb[:, ot:ot + 1],
                )
                eng = nc.sync if ot % 2 == 0 else nc.scalar
                eng.dma_start(out=out_d[ts(ot, P), ts(j, width)],
                              in_=out_sb[:])

            def emit_half(ot, ps, tb):
                emit_part(ot, ps, tb, 512)

            for a in range(A):
                for tb in range(TB):
                    emit_half(a, ps_a[a], tb)

            # Phase B: k-contiguous, tb-outer so each half's ACT+store
            # overlaps the other half's matmuls; W slabs prefetched on the
            # gpsimd SWDGE queue (pool-depth throttled).
            for ot in range(A, OT):
                if ot not in wb:
                    load_wb(ot, nc.gpsimd)
                wt = wb[ot]
                ps = pspool.tile([P, T], F32, tag="ps", name=f"ps{ot}")
                # taper the final o-tile so only a 256-wide ACT+store
                # trails the last matmul
                spans = ([(0, 512), (512, 256), (768, 256)]
                         if ot == OT - 1 else [(0, 512), (512, 512)])
                for off, width in spans:
                    for kt in range(KT):
                        nc.tensor.matmul(
                            ps[:, off:off + width],
                            lhsT=wt[:, kt, :],
                            rhs=x_tb[kt][off // 512][:, off % 512:off % 512 + width]
                                if width != 512 else x_tb[kt][off // 512],
                            start=(kt == 0),
                            stop=(kt == KT - 1),
                        )
                    out_sb = opool.tile([P, width], BF16)
                    nc.scalar.activation(
                        out_sb[:],
                        ps[:, off:off + width],
                        mybir.ActivationFunctionType.Identity,
                        bias=bias_sb[:, ot:ot + 1],
                    )
                    eng = nc.sync if ot % 2 == 0 else nc.scalar
                    eng.dma_start(out=out_d[ts(ot, P), off:off + width],
                                  in_=out_sb[:])

    nc.compile()
    return nc


def _prep_core_inputs(x2d, W_eff, bias, K, O, T, n_cores):
    """Host-side layout prep: shard tokens, make partition-major layouts."""
    KT, OT = K // P, O // P
    A, CH = A_OT, KT // KC
    w_bf = W_eff.astype(ml_dtypes.bfloat16)
    w_dev = np.ascontiguousarray(
        w_bf.reshape(OT, P, KT, P).transpose(0, 3, 2, 1)
    )  # [ot, k, kt, o]
    # [c, k, a, kc, o] from W_eff[a*128+o, (c*KC+kc)*128+k]
    wa_dev = np.ascontiguousarray(
        w_bf[:A * P].reshape(A, P, CH, KC, P).transpose(2, 4, 0, 3, 1)
    )
    bias_dev = np.ascontiguousarray(bias.reshape(OT, P).T)  # [o(part), ot]
    # first-packet weights: kt0+kt1 blocks for the A phase-A o-tiles,
    # laid out [k, (kt*A + a)*P + o] = W_eff[a*128+o, kt*128+k]
    wpkt = np.ascontiguousarray(
        w_bf[:A * P, :2 * P].reshape(A, P, 2, P).transpose(3, 2, 0, 1)
        .reshape(P, 2 * A * P)
    )
    in_maps = []
    for c in range(n_cores):
        xs = x2d[c * T:(c + 1) * T]  # [T, K]
        x_dev = np.ascontiguousarray(
            xs.astype(ml_dtypes.bfloat16).reshape(T, KT, P).transpose(2, 1, 0)
        )  # [k, kt, t]
        pkt_dev = np.ascontiguousarray(
            np.concatenate([wpkt, x_dev[:, 0, :512], x_dev[:, 1, :512]],
                           axis=1)
        )
        in_maps.append({"x": x_dev, "w": w_dev, "wa": wa_dev,
                        "bias": bias_dev, "pkt": pkt_dev})
    return in_maps


def kernel(x, W, bias, lora_A, lora_B, trace=False, _nc_cache={}):
    x = np.asarray(x, np.float32)
    W = np.asarray(W, np.float32)
    bias = np.asarray(bias, np.float32)
    lora_A = np.asarray(lora_A, np.float32)
    lora_B = np.asarray(lora_B, np.float32)
    B, S, D_in = x.shape
    D_out = bias.shape[0]
    T_total = B * S
    T = T_total // N_CORES

    W_eff = host_fold_lora(W, bias, lora_A, lora_B)
    x2d = np.ascontiguousarray(x.reshape(T_total, D_in))

    key = (D_in, D_out, T)
    if key not in _nc_cache:
        _nc_cache[key] = build_bass(D_in, D_out, T)
    nc = _nc_cache[key]

    in_maps = _prep_core_inputs(x2d, W_eff, bias, D_in, D_out, T, N_CORES)
    res = run_bass_kernel_spmd(nc, in_maps, list(range(N_CORES)), trace=trace)

    out = np.empty((T_total, D_out), dtype=np.float32)
    for c in range(N_CORES):
        out[c * T:(c + 1) * T] = res.results[c]["out"].astype(np.float32).T
    out = out.reshape(B, S, D_out)
    if trace:
        return out, res
    return out


# revision 29
# speedup vs baseline: 1.0020x; 1.0020x over previous
"""Trainium2 Bass kernel for nn_MultiHeadMuonLoRALinear.

Math: out = x @ W^T + bias + sum_h alpha_h * x @ M_h^T, where
M_h = newtonschulz5(B_h @ A_h) and G_h = B_h @ A_h has rank hr=4.

Key algebraic identity: with G = B A (rank hr), every Newton-Schulz
iterate stays in the same row/column space, so X_k = B C_k A for an
hr x hr matrix C_k:
    C_0 = I / (||G||_F + eps),  ||G||_F^2 = tr((B^T B)(A A^T))
    C'  = a C + b (C P C^T) Q C + c (C P C^T Q)^2 C,  P = A A^T, Q = B^T B
Therefore M_h = B_h C_h A_h and the whole LoRA branch collapses to a
rank-16 update:  delta = sum_h alpha_h B_h C_h A_h,  out = x @ (W + delta)^T + bias.

The device kernel computes the single large GEMM (data-parallel over
tokens across 8 cores) with the rank-16 delta folded into W on the
host (0.2% of total FLOPs).

Device strategy (per core, T=1024 tokens, K=O=4096):
  - All-bf16 GEMM, f32 PSUM. The PE streams 1 moving column/cycle for
    every dtype >= bf16 (fp8 DoubleRow is the only 2x mode but fails
    the 2e-2 accuracy budget), so the per-core compute floor is
    32*32*2 matmuls x 512 columns = 437us. bf16 halves DMA vs fp32r
    and, critically, enables FWL (fast weight load), which fp32r's
    FP32-HIGH mode disables — LDWEIGHTS becomes fully hidden and the
    steady matmul issue rate drops from 233ns to 216ns.
  - Phase A k-sweeps the first A=4 o-tiles across the four double-bank
    PSUM groups, consuming x tiles in their DMA arrival order. The x
    tiles and phase-A weight chunks are spread across all three DMA
    queues (sync/scalar HWDGE ~100 GB/s each, gpsimd SWDGE ~210 GB/s)
    in a just-in-time order so the PE never starves while x streams.
  - Phase B runs the remaining o-tiles k-contiguous, tb-outer so each
    half's activation+store overlaps the other half's matmuls; W slabs
    are prefetched on the gpsimd queue, throttled by the pool depth.
  - A short PE warmup (dep-free matmuls) covers the fixed ~9us NEFF
    preamble + first-tile DMA window so the HAM clock gate is released
    before the first real matmul.
"""

import numpy as np
import ml_dtypes

import concourse.bass as bass
import concourse.bacc as bacc
import concourse.mybir as mybir
import concourse.tile as tile
from concourse.bass import ts
from concourse.bass_utils import run_bass_kernel_spmd

N_HEADS = 4
NS_STEPS = 5
NS_EPS = 1e-7
NS_A, NS_B, NS_C = 3.4445, -4.775, 2.0315

N_CORES = 8
P = 128

F32 = mybir.dt.float32
BF16 = mybir.dt.bfloat16

A_OT = 4       # phase-A o-tiles (PSUM: 4 groups x 2 banks = all 8 banks)
KC = 4         # k-tiles per phase-A weight chunk
WARMUP = 11


def host_fold_lora(W, bias, lora_A, lora_B):
    """Collapse the per-head Newton-Schulz into hr x hr space (float64)
    and return W_eff = W + sum_h alpha_h B_h C_h A_h (float32)."""
    r, D_in = lora_A.shape
    D_out = lora_B.shape[0]
    hr = r // N_HEADS
    Ah = lora_A.reshape(N_HEADS, hr, D_in).astype(np.float64)
    Bh = lora_B.reshape(D_out, N_HEADS, hr).transpose(1, 0, 2).astype(np.float64)

    AT = np.zeros((r, D_in))   # rows: alpha-weighted C_h A_h per head
    BT = np.zeros((D_out, r))  # cols: B_h per head
    for h in range(N_HEADS):
        A = Ah[h]
        B = Bh[h]
        Pm = A @ A.T
        Qm = B.T @ B
        fro = np.sqrt(np.trace(Qm @ Pm))
        C = np.eye(hr) / (fro + NS_EPS)
        for _ in range(NS_STEPS):
            D = C @ Pm @ C.T
            E = D @ Qm
            C = NS_A * C + NS_B * (E @ C) + NS_C * (E @ (E @ C))
        AT[h * hr:(h + 1) * hr] = fro * (C @ A)
        BT[:, h * hr:(h + 1) * hr] = B
    delta = BT @ AT
    return (W.astype(np.float64) + delta).astype(np.float32)


def build_bass(K, O, T):
    """Per-core SPMD program: outT[O, T] = (x W_eff^T + bias)^T for this
    core's token shard, all-bf16 GEMM with f32 PSUM accumulation.

    DRAM layouts (host-prepared):
      x:    [128, K//128, T] bf16   x_dev[k, kt, t] = x_shard[t, kt*128 + k]
      w:    [O//128, 128, K//128, 128] bf16  w_dev[ot, k, kt, o] = W_eff[ot*128+o, kt*128+k]
      wa:   [CH, 128, A, KC, 128] bf16  phase-A chunk groups:
            wa_dev[c, k, a, kc, o] = W_eff[a*128+o, (c*KC+kc)*128+k]
      bias: [128, O//128] f32       bias_dev[o, ot] = bias[ot*128 + o]
      out:  [O, T] bf16             outT
    """
    KT, OT = K // P, O // P
    A = A_OT
    CH = KT // KC
    TB = T // 512
    nc = bacc.Bacc()

    x_d = nc.declare_dram_parameter("x", [P, KT, T], BF16, isOutput=False)
    w_d = nc.declare_dram_parameter("w", [OT, P, KT, P], BF16, isOutput=False)
    wa_d = nc.declare_dram_parameter("wa", [CH, P, A, KC, P], BF16,
                                     isOutput=False)
    pkt_d = nc.declare_dram_parameter("pkt", [P, 2 * A * P + 512], BF16,
                                      isOutput=False)
    b_d = nc.declare_dram_parameter("bias", [P, OT], F32, isOutput=False)
    out_d = nc.declare_dram_parameter("out", [O, T], BF16, isOutput=True)

    with tile.TileContext(nc) as tc:
        with (
            tc.tile_pool(name="xpool", bufs=1) as xpool,
            tc.tile_pool(name="cpool", bufs=1) as cpool,
            tc.tile_pool(name="wapool", bufs=1) as wapool,
            tc.tile_pool(name="wbpool", bufs=6) as wbpool,
            tc.tile_pool(name="opool", bufs=6) as opool,
            tc.tile_pool(name="pspool", bufs=4, space="PSUM") as pspool,
        ):
            # x access: x_tb[kt][tb] -> [128, 512] AP for that k-tile half.
            x_tb = [None] * KT

            def load_x(kt, engine):
                xt = xpool.tile([P, T], BF16, tag=f"x{kt}", name=f"x{kt}")
                engine.dma_start(out=xt[:], in_=x_d[:, kt, :])
                x_tb[kt] = [xt[:, ts(tb, 512)] for tb in range(TB)]

            def load_x_halves(kt, engine):
                # Two half-tiles so the first 512 tokens land ~2us sooner;
                # used for the k-tiles that gate the phase-A start.
                aps = []
                for tb in range(TB):
                    xt = xpool.tile([P, 512], BF16, tag=f"x{kt}_{tb}",
                                    name=f"x{kt}_{tb}")
                    engine.dma_start(out=xt[:],
                                     in_=x_d[:, kt, ts(tb, 512)])
                    aps.append(xt[:])
                x_tb[kt] = aps

            def load_x_batch(k0, k1, engine):
                n = k1 - k0
                xt = xpool.tile([P, n, T], BF16, tag=f"xb{k0}",
                                name=f"xb{k0}")
                engine.dma_start(out=xt[:], in_=x_d[:, k0:k1, :])
                for kt in range(k0, k1):
                    x_tb[kt] = [xt[:, kt - k0, ts(tb, 512)]
                                for tb in range(TB)]

            # Phase-A weights: wlhsT(a, kt) -> [128, 128] stationary AP.
            wa_group = {}    # c -> tile [P, A, KC, P]
            # First packet: kt0+kt1 weights for all A o-tiles plus the
            # first 512-token half of x[kt0], as ONE dma on the fast
            # gpsimd queue so the phase-A start is deterministic (~12us)
            # instead of racing the contended HWDGE queues.
            pkt = [None]

            def load_pkt(engine):
                wt = wapool.tile([P, 2 * A * P + 512], BF16, tag="pkt",
                                 name="pkt")
                engine.dma_start(out=wt[:], in_=pkt_d[:])
                pkt[0] = wt

            def load_wa_group(c, engine):
                wt = wapool.tile([P, A, KC, P], BF16, tag=f"wag{c}",
                                 name=f"wag{c}")
                engine.dma_start(out=wt[:], in_=wa_d[c])
                wa_group[c] = wt

            def wa_lhsT(a, kt):
                if kt < 2 and pkt[0] is not None:
                    return pkt[0][:, (kt * A + a) * P:(kt * A + a + 1) * P]
                return wa_group[kt // KC][:, a, kt % KC, :]

            wb = {}          # ot -> slab tile [P, KT, P]

            def load_wb(ot, engine):
                wt = wbpool.tile([P, KT, P], BF16, tag="wb", name=f"w{ot}")
                engine.dma_start(out=wt[:], in_=w_d[ot])
                wb[ot] = wt

            bias_sb = cpool.tile([P, OT], F32)

            # ---- queue programs (emission order == per-engine queue order)
            # Just-in-time supply: phase A consumes (x[kt], wa chunk) pairs
            # at ~1.73us per k-tile from t~14us; each item below lands
            # (at ~100 GB/s HWDGE / ~210 GB/s SWDGE) ahead of its deadline.
            # sync HWDGE (~55 GB/s while contended early, ~100 after):
            # x0's first half rides in the gpsimd first packet; load only
            # its second half here.
            x0h1 = xpool.tile([P, 512], BF16, tag="x0_1", name="x0_1")
            nc.sync.dma_start(out=x0h1[:], in_=x_d[:, 0, ts(1, 512)])
            for kt in (2, 4, 6, 12, 14):
                load_x(kt, nc.sync)
            load_x_batch(16, 20, nc.sync)
            load_x_batch(24, 28, nc.sync)
            # scalar HWDGE:
            load_x_halves(1, nc.scalar)
            for kt in (3, 5, 7, 13, 15):
                load_x(kt, nc.scalar)
            load_x_batch(20, 24, nc.scalar)
            load_x_batch(28, 32, nc.scalar)
            nc.scalar.dma_start(out=bias_sb[:], in_=b_d[:])
            # gpsimd SWDGE (~190 GB/s, ~1.3us fixed/item): all phase-A
            # weight chunk groups plus the mid x batch, in deadline order.
            load_pkt(nc.gpsimd)
            load_wa_group(0, nc.gpsimd)
            load_wa_group(1, nc.gpsimd)
            load_x_batch(8, 12, nc.gpsimd)
            load_wa_group(2, nc.gpsimd)
            load_wa_group(3, nc.gpsimd)
            load_wa_group(4, nc.gpsimd)
            load_wa_group(5, nc.gpsimd)
            load_wa_group(6, nc.gpsimd)
            load_wa_group(7, nc.gpsimd)
            load_wb(4, nc.gpsimd)
            load_wb(5, nc.gpsimd)
            x_tb[0] = [pkt[0][:, 2 * A * P:2 * A * P + 512], x0h1[:]]

            # PE warmup across the preamble + first-tile DMA window.
            wu_src = cpool.tile([P, 512], BF16, name="wu_src")
            nc.vector.memset(wu_src[:], 0.0)
            wu_ps = pspool.tile([P, T], F32, tag="ps", name="wu_ps")
            for i in range(WARMUP):
                nc.tensor.matmul(
                    wu_ps[:, :512], lhsT=wu_src[:, :P], rhs=wu_src[:],
                    start=(i == 0), stop=(i == WARMUP - 1),
                )

            # Phase A: k-outer sweep over the first A o-tiles in parallel
            # PSUM groups. o-tile order 2,3,0,1 at each k so the first
            # matmuls wait on the earliest-arriving weight chunks.
            ps_a = [pspool.tile([P, T], F32, tag="ps", name=f"psA{a}")
                    for a in range(A)]
            for kt in range(KT):
                for a in range(A):
                    for tb in range(TB):
                        nc.tensor.matmul(
                            ps_a[a][:, ts(tb, 512)],
                            lhsT=wa_lhsT(a, kt),
                            rhs=x_tb[kt][tb],
                            start=(kt == 0),
                            stop=(kt == KT - 1),
                        )

            def emit_part(ot, ps, j, width):
                out_sb = opool.tile([P, width], BF16)
                nc.scalar.activation(
                    out_sb[:],
                    ps[:, ts(j, width)],
                    mybir.ActivationFunctionType.Identity,
                    bias=bias_sb[:, ot:ot + 1],
                )
                eng = nc.sync if ot % 2 == 0 else nc.scalar
                eng.dma_start(out=out_d[ts(ot, P), ts(j, width)],
                              in_=out_sb[:])

            def emit_half(ot, ps, tb):
                emit_part(ot, ps, tb, 512)

            for a in range(A):
                for tb in range(TB):
                    emit_half(a, ps_a[a], tb)

            # Phase B: k-contiguous, tb-outer so each half's ACT+store
            # overlaps the other half's matmuls; W slabs prefetched on the
            # gpsimd SWDGE queue (pool-depth throttled).
            for ot in range(A, OT):
                if ot not in wb:
                    load_wb(ot, nc.gpsimd)
                wt = wb[ot]
                ps = pspool.tile([P, T], F32, tag="ps", name=f"ps{ot}")
                # taper the final o-tile so only a 256-wide ACT+store
                # trails the last matmul
                spans = ([(0, 512), (512, 256), (768, 256)]
                         if ot == OT - 1 else [(0, 512), (512, 512)])
                for off, width in spans:
                    for kt in range(KT):
                        nc.tensor.matmul(
                            ps[:, off:off + width],
                            lhsT=wt[:, kt, :],
                            rhs=x_tb[kt][off // 512][:, off % 512:off % 512 + width]
                                if width != 512 else x_tb[kt][off // 512],
                            start=(kt == 0),
                            stop=(kt == KT - 1),
                        )
                    out_sb = opool.tile([P, width], BF16)
                    nc.scalar.activation(
                        out_sb[:],
                        ps[:, off:off + width],
                        mybir.ActivationFunctionType.Identity,
                        bias=bias_sb[:, ot:ot + 1],
                    )
                    eng = nc.sync if ot % 2 == 0 else nc.scalar
                    eng.dma_start(out=out_d[ts(ot, P), off:off + width],
                                  in_=out_sb[:])

    nc.compile()
    return nc


def _prep_core_inputs(x2d, W_eff, bias, K, O, T, n_cores):
    """Host-side layout prep: shard tokens, make partition-major layouts."""
    KT, OT = K // P, O // P
    A, CH = A_OT, KT // KC
    w_bf = W_eff.astype(ml_dtypes.bfloat16)
    w_dev = np.ascontiguousarray(
        w_bf.reshape(OT, P, KT, P).transpose(0, 3, 2, 1)
    )  # [ot, k, kt, o]
    # [c, k, a, kc, o] from W_eff[a*128+o, (c*KC+kc)*128+k]
    wa_dev = np.ascontiguousarray(
        w_bf[:A * P].reshape(A, P, CH, KC, P).transpose(2, 4, 0, 3, 1)
    )
    bias_dev = np.ascontiguousarray(bias.reshape(OT, P).T)  # [o(part), ot]
    # first-packet weights: kt0+kt1 blocks for the A phase-A o-tiles,
    # laid out [k, (kt*A + a)*P + o] = W_eff[a*128+o, kt*128+k]
    wpkt = np.ascontiguousarray(
        w_bf[:A * P, :2 * P].reshape(A, P, 2, P).transpose(3, 2, 0, 1)
        .reshape(P, 2 * A * P)
    )
    in_maps = []
    for c in range(n_cores):
        xs = x2d[c * T:(c + 1) * T]  # [T, K]
        x_dev = np.ascontiguousarray(
            xs.astype(ml_dtypes.bfloat16).reshape(T, KT, P).transpose(2, 1, 0)
        )  # [k, kt, t]
        pkt_dev = np.ascontiguousarray(
            np.concatenate([wpkt, x_dev[:, 0, :512]], axis=1)
        )
        in_maps.append({"x": x_dev, "w": w_dev, "wa": wa_dev,
                        "bias": bias_dev, "pkt": pkt_dev})
    return in_maps


def kernel(x, W, bias, lora_A, lora_B, trace=False, _nc_cache={}):
    x = np.asarray(x, np.float32)
    W = np.asarray(W, np.float32)
    bias = np.asarray(bias, np.float32)
    lora_A = np.asarray(lora_A, np.float32)
    lora_B = np.asarray(lora_B, np.float32)
    B, S, D_in = x.shape
    D_out = bias.shape[0]
    T_total = B * S
    T = T_total // N_CORES

    W_eff = host_fold_lora(W, bias, lora_A, lora_B)
    x2d = np.ascontiguousarray(x.reshape(T_total, D_in))

    key = (D_in, D_out, T)
    if key not in _nc_cache:
        _nc_cache[key] = build_bass(D_in, D_out, T)
    nc = _nc_cache[key]

    in_maps = _prep_core_inputs(x2d, W_eff, bias, D_in, D_out, T, N_CORES)
    res = run_bass_kernel_spmd(nc, in_maps, list(range(N_CORES)), trace=trace)

    out = np.empty((T_total, D_out), dtype=np.float32)
    for c in range(N_CORES):
        out[c * T:(c + 1) * T] = res.results[c]["out"].astype(np.float32).T
    out = out.reshape(B, S, D_out)
    if trace:
        return out, res
    return out


# revision 30
# speedup vs baseline: 1.0063x; 1.0043x over previous
"""Trainium2 Bass kernel for nn_MultiHeadMuonLoRALinear.

Math: out = x @ W^T + bias + sum_h alpha_h * x @ M_h^T, where
M_h = newtonschulz5(B_h @ A_h) and G_h = B_h @ A_h has rank hr=4.

Key algebraic identity: with G = B A (rank hr), every Newton-Schulz
iterate stays in the same row/column space, so X_k = B C_k A for an
hr x hr matrix C_k:
    C_0 = I / (||G||_F + eps),  ||G||_F^2 = tr((B^T B)(A A^T))
    C'  = a C + b (C P C^T) Q C + c (C P C^T Q)^2 C,  P = A A^T, Q = B^T B
Therefore M_h = B_h C_h A_h and the whole LoRA branch collapses to a
rank-16 update:  delta = sum_h alpha_h B_h C_h A_h,  out = x @ (W + delta)^T + bias.

The device kernel computes the single large GEMM (data-parallel over
tokens across 8 cores) with the rank-16 delta folded into W on the
host (0.2% of total FLOPs).

Device strategy (per core, T=1024 tokens, K=O=4096):
  - All-bf16 GEMM, f32 PSUM. The PE streams 1 moving column/cycle for
    every dtype >= bf16 (fp8 DoubleRow is the only 2x mode but fails
    the 2e-2 accuracy budget), so the per-core compute floor is
    32*32*2 matmuls x 512 columns = 437us. bf16 halves DMA vs fp32r
    and, critically, enables FWL (fast weight load), which fp32r's
    FP32-HIGH mode disables — LDWEIGHTS becomes fully hidden and the
    steady matmul issue rate drops from 233ns to 216ns.
  - Phase A k-sweeps the first A=4 o-tiles across the four double-bank
    PSUM groups, consuming x tiles in their DMA arrival order. The x
    tiles and phase-A weight chunks are spread across all three DMA
    queues (sync/scalar HWDGE ~100 GB/s each, gpsimd SWDGE ~210 GB/s)
    in a just-in-time order so the PE never starves while x streams.
  - Phase B runs the remaining o-tiles k-contiguous, tb-outer so each
    half's activation+store overlaps the other half's matmuls; W slabs
    are prefetched on the gpsimd queue, throttled by the pool depth.
  - A short PE warmup (dep-free matmuls) covers the fixed ~9us NEFF
    preamble + first-tile DMA window so the HAM clock gate is released
    before the first real matmul.
"""

import numpy as np
import ml_dtypes

import concourse.bass as bass
import concourse.bacc as bacc
import concourse.mybir as mybir
import concourse.tile as tile
from concourse.bass import ts
from concourse.bass_utils import run_bass_kernel_spmd

N_HEADS = 4
NS_STEPS = 5
NS_EPS = 1e-7
NS_A, NS_B, NS_C = 3.4445, -4.775, 2.0315

N_CORES = 8
P = 128

F32 = mybir.dt.float32
BF16 = mybir.dt.bfloat16

A_OT = 4       # phase-A o-tiles (PSUM: 4 groups x 2 banks = all 8 banks)
KC = 4         # k-tiles per phase-A weight chunk
WARMUP = 11


def host_fold_lora(W, bias, lora_A, lora_B):
    """Collapse the per-head Newton-Schulz into hr x hr space (float64)
    and return W_eff = W + sum_h alpha_h B_h C_h A_h (float32)."""
    r, D_in = lora_A.shape
    D_out = lora_B.shape[0]
    hr = r // N_HEADS
    Ah = lora_A.reshape(N_HEADS, hr, D_in).astype(np.float64)
    Bh = lora_B.reshape(D_out, N_HEADS, hr).transpose(1, 0, 2).astype(np.float64)

    AT = np.zeros((r, D_in))   # rows: alpha-weighted C_h A_h per head
    BT = np.zeros((D_out, r))  # cols: B_h per head
    for h in range(N_HEADS):
        A = Ah[h]
        B = Bh[h]
        Pm = A @ A.T
        Qm = B.T @ B
        fro = np.sqrt(np.trace(Qm @ Pm))
        C = np.eye(hr) / (fro + NS_EPS)
        for _ in range(NS_STEPS):
            D = C @ Pm @ C.T
            E = D @ Qm
            C = NS_A * C + NS_B * (E @ C) + NS_C * (E @ (E @ C))
        AT[h * hr:(h + 1) * hr] = fro * (C @ A)
        BT[:, h * hr:(h + 1) * hr] = B
    delta = BT @ AT
    return (W.astype(np.float64) + delta).astype(np.float32)


def build_bass(K, O, T):
    """Per-core SPMD program: outT[O, T] = (x W_eff^T + bias)^T for this
    core's token shard, all-bf16 GEMM with f32 PSUM accumulation.

    DRAM layouts (host-prepared):
      x:    [128, K//128, T] bf16   x_dev[k, kt, t] = x_shard[t, kt*128 + k]
      w:    [O//128, 128, K//128, 128] bf16  w_dev[ot, k, kt, o] = W_eff[ot*128+o, kt*128+k]
      wa:   [CH, 128, A, KC, 128] bf16  phase-A chunk groups:
            wa_dev[c, k, a, kc, o] = W_eff[a*128+o, (c*KC+kc)*128+k]
      bias: [128, O//128] f32       bias_dev[o, ot] = bias[ot*128 + o]
      out:  [O, T] bf16             outT
    """
    KT, OT = K // P, O // P
    A = A_OT
    CH = KT // KC
    TB = T // 512
    nc = bacc.Bacc()

    x_d = nc.declare_dram_parameter("x", [P, KT, T], BF16, isOutput=False)
    w_d = nc.declare_dram_parameter("w", [OT, P, KT, P], BF16, isOutput=False)
    wa_d = nc.declare_dram_parameter("wa", [CH, P, A, KC, P], BF16,
                                     isOutput=False)
    pkt_d = nc.declare_dram_parameter("pkt", [P, 2 * A * P + 512], BF16,
                                      isOutput=False)
    b_d = nc.declare_dram_parameter("bias", [P, OT], F32, isOutput=False)
    out_d = nc.declare_dram_parameter("out", [O, T], BF16, isOutput=True)

    with tile.TileContext(nc) as tc:
        with (
            tc.tile_pool(name="xpool", bufs=1) as xpool,
            tc.tile_pool(name="cpool", bufs=1) as cpool,
            tc.tile_pool(name="wapool", bufs=1) as wapool,
            tc.tile_pool(name="wbpool", bufs=6) as wbpool,
            tc.tile_pool(name="opool", bufs=6) as opool,
            tc.tile_pool(name="pspool", bufs=4, space="PSUM") as pspool,
        ):
            # x access: x_tb[kt][tb] -> [128, 512] AP for that k-tile half.
            x_tb = [None] * KT

            def load_x(kt, engine):
                xt = xpool.tile([P, T], BF16, tag=f"x{kt}", name=f"x{kt}")
                engine.dma_start(out=xt[:], in_=x_d[:, kt, :])
                x_tb[kt] = [xt[:, ts(tb, 512)] for tb in range(TB)]

            def load_x_halves(kt, engine):
                # Two half-tiles so the first 512 tokens land ~2us sooner;
                # used for the k-tiles that gate the phase-A start.
                aps = []
                for tb in range(TB):
                    xt = xpool.tile([P, 512], BF16, tag=f"x{kt}_{tb}",
                                    name=f"x{kt}_{tb}")
                    engine.dma_start(out=xt[:],
                                     in_=x_d[:, kt, ts(tb, 512)])
                    aps.append(xt[:])
                x_tb[kt] = aps

            def load_x_batch(k0, k1, engine):
                n = k1 - k0
                xt = xpool.tile([P, n, T], BF16, tag=f"xb{k0}",
                                name=f"xb{k0}")
                engine.dma_start(out=xt[:], in_=x_d[:, k0:k1, :])
                for kt in range(k0, k1):
                    x_tb[kt] = [xt[:, kt - k0, ts(tb, 512)]
                                for tb in range(TB)]

            # Phase-A weights: wlhsT(a, kt) -> [128, 128] stationary AP.
            wa_group = {}    # c -> tile [P, A, KC, P]
            # First packet: kt0+kt1 weights for all A o-tiles plus the
            # first 512-token half of x[kt0], as ONE dma on the fast
            # gpsimd queue so the phase-A start is deterministic (~12us)
            # instead of racing the contended HWDGE queues.
            pkt = [None]

            def load_pkt(engine):
                wt = wapool.tile([P, 2 * A * P + 512], BF16, tag="pkt",
                                 name="pkt")
                engine.dma_start(out=wt[:], in_=pkt_d[:])
                pkt[0] = wt

            def load_wa_group(c, engine):
                wt = wapool.tile([P, A, KC, P], BF16, tag=f"wag{c}",
                                 name=f"wag{c}")
                engine.dma_start(out=wt[:], in_=wa_d[c])
                wa_group[c] = wt

            def wa_lhsT(a, kt):
                if kt < 2 and pkt[0] is not None:
                    return pkt[0][:, (kt * A + a) * P:(kt * A + a + 1) * P]
                return wa_group[kt // KC][:, a, kt % KC, :]

            wb = {}          # ot -> slab tile [P, KT, P]

            def load_wb(ot, engine):
                wt = wbpool.tile([P, KT, P], BF16, tag="wb", name=f"w{ot}")
                engine.dma_start(out=wt[:], in_=w_d[ot])
                wb[ot] = wt

            bias_sb = cpool.tile([P, OT], F32)

            # ---- queue programs (emission order == per-engine queue order)
            # Just-in-time supply: phase A consumes (x[kt], wa chunk) pairs
            # at ~1.73us per k-tile from t~14us; each item below lands
            # (at ~100 GB/s HWDGE / ~210 GB/s SWDGE) ahead of its deadline.
            # sync HWDGE (~55 GB/s while contended early, ~100 after):
            # x0's first half rides in the gpsimd first packet; load only
            # its second half here.
            x0h1 = xpool.tile([P, 512], BF16, tag="x0_1", name="x0_1")
            nc.sync.dma_start(out=x0h1[:], in_=x_d[:, 0, ts(1, 512)])
            for kt in (2, 3, 4, 6, 12, 14):
                load_x(kt, nc.sync)
            load_x_batch(16, 20, nc.sync)
            load_x_batch(24, 28, nc.sync)
            # scalar HWDGE:
            load_x_halves(1, nc.scalar)
            for kt in (5, 7, 13, 15):
                load_x(kt, nc.scalar)
            load_x_batch(20, 24, nc.scalar)
            load_x_batch(28, 32, nc.scalar)
            nc.scalar.dma_start(out=bias_sb[:], in_=b_d[:])
            # gpsimd SWDGE (~190 GB/s, ~1.3us fixed/item): all phase-A
            # weight chunk groups plus the mid x batch, in deadline order.
            load_pkt(nc.gpsimd)
            load_wa_group(0, nc.gpsimd)
            load_wa_group(1, nc.gpsimd)
            load_x_batch(8, 12, nc.gpsimd)
            load_wa_group(2, nc.gpsimd)
            load_wa_group(3, nc.gpsimd)
            load_wa_group(4, nc.gpsimd)
            load_wa_group(5, nc.gpsimd)
            load_wa_group(6, nc.gpsimd)
            load_wa_group(7, nc.gpsimd)
            load_wb(4, nc.gpsimd)
            load_wb(5, nc.gpsimd)
            x_tb[0] = [pkt[0][:, 2 * A * P:2 * A * P + 512], x0h1[:]]

            # PE warmup across the preamble + first-tile DMA window.
            wu_src = cpool.tile([P, 512], BF16, name="wu_src")
            nc.vector.memset(wu_src[:], 0.0)
            wu_ps = pspool.tile([P, T], F32, tag="ps", name="wu_ps")
            for i in range(WARMUP):
                nc.tensor.matmul(
                    wu_ps[:, :512], lhsT=wu_src[:, :P], rhs=wu_src[:],
                    start=(i == 0), stop=(i == WARMUP - 1),
                )

            # Phase A: k-outer sweep over the first A o-tiles in parallel
            # PSUM groups. o-tile order 2,3,0,1 at each k so the first
            # matmuls wait on the earliest-arriving weight chunks.
            ps_a = [pspool.tile([P, T], F32, tag="ps", name=f"psA{a}")
                    for a in range(A)]
            for kt in range(KT):
                for a in range(A):
                    for tb in range(TB):
                        nc.tensor.matmul(
                            ps_a[a][:, ts(tb, 512)],
                            lhsT=wa_lhsT(a, kt),
                            rhs=x_tb[kt][tb],
                            start=(kt == 0),
                            stop=(kt == KT - 1),
                        )

            def emit_part(ot, ps, j, width):
                out_sb = opool.tile([P, width], BF16)
                nc.scalar.activation(
                    out_sb[:],
                    ps[:, ts(j, width)],
                    mybir.ActivationFunctionType.Identity,
                    bias=bias_sb[:, ot:ot + 1],
                )
                eng = nc.sync if ot % 2 == 0 else nc.scalar
                eng.dma_start(out=out_d[ts(ot, P), ts(j, width)],
                              in_=out_sb[:])

            def emit_half(ot, ps, tb):
                emit_part(ot, ps, tb, 512)

            for a in range(A):
                for tb in range(TB):
                    emit_half(a, ps_a[a], tb)

            # Phase B: k-contiguous, tb-outer so each half's ACT+store
            # overlaps the other half's matmuls; W slabs prefetched on the
            # gpsimd SWDGE queue (pool-depth throttled).
            for ot in range(A, OT):
                if ot not in wb:
                    load_wb(ot, nc.gpsimd)
                wt = wb[ot]
                # taper the final o-tile so only a 256-wide ACT+store
                # trails the last matmul; each group gets its own PSUM
                # tile so a group's matmuls don't false-depend on the
                # previous group's ACT (tile-granular tracking)
                spans = ([(0, 512), (512, 256), (768, 256)]
                         if ot == OT - 1 else [(0, 512), (512, 512)])
                for off, width in spans:
                    ps = pspool.tile([P, width], F32, tag="ps",
                                     name=f"ps{ot}_{off}")
                    for kt in range(KT):
                        nc.tensor.matmul(
                            ps[:],
                            lhsT=wt[:, kt, :],
                            rhs=x_tb[kt][off // 512][:, off % 512:off % 512 + width]
                                if width != 512 else x_tb[kt][off // 512],
                            start=(kt == 0),
                            stop=(kt == KT - 1),
                        )
                    out_sb = opool.tile([P, width], BF16)
                    nc.scalar.activation(
                        out_sb[:],
                        ps[:],
                        mybir.ActivationFunctionType.Identity,
                        bias=bias_sb[:, ot:ot + 1],
                    )
                    eng = nc.sync if ot % 2 == 0 else nc.scalar
                    eng.dma_start(out=out_d[ts(ot, P), off:off + width],
                                  in_=out_sb[:])

    nc.compile()
    return nc


def _prep_core_inputs(x2d, W_eff, bias, K, O, T, n_cores):
    """Host-side layout prep: shard tokens, make partition-major layouts."""
    KT, OT = K // P, O // P
    A, CH = A_OT, KT // KC
    w_bf = W_eff.astype(ml_dtypes.bfloat16)
    w_dev = np.ascontiguousarray(
        w_bf.reshape(OT, P, KT, P).transpose(0, 3, 2, 1)
    )  # [ot, k, kt, o]
    # [c, k, a, kc, o] from W_eff[a*128+o, (c*KC+kc)*128+k]
    wa_dev = np.ascontiguousarray(
        w_bf[:A * P].reshape(A, P, CH, KC, P).transpose(2, 4, 0, 3, 1)
    )
    bias_dev = np.ascontiguousarray(bias.reshape(OT, P).T)  # [o(part), ot]
    # first-packet weights: kt0+kt1 blocks for the A phase-A o-tiles,
    # laid out [k, (kt*A + a)*P + o] = W_eff[a*128+o, kt*128+k]
    wpkt = np.ascontiguousarray(
        w_bf[:A * P, :2 * P].reshape(A, P, 2, P).transpose(3, 2, 0, 1)
        .reshape(P, 2 * A * P)
    )
    in_maps = []
    for c in range(n_cores):
        xs = x2d[c * T:(c + 1) * T]  # [T, K]
        x_dev = np.ascontiguousarray(
            xs.astype(ml_dtypes.bfloat16).reshape(T, KT, P).transpose(2, 1, 0)
        )  # [k, kt, t]
        pkt_dev = np.ascontiguousarray(
            np.concatenate([wpkt, x_dev[:, 0, :512]], axis=1)
        )
        in_maps.append({"x": x_dev, "w": w_dev, "wa": wa_dev,
                        "bias": bias_dev, "pkt": pkt_dev})
    return in_maps


def kernel(x, W, bias, lora_A, lora_B, trace=False, _nc_cache={}):
    x = np.asarray(x, np.float32)
    W = np.asarray(W, np.float32)
    bias = np.asarray(bias, np.float32)
    lora_A = np.asarray(lora_A, np.float32)
    lora_B = np.asarray(lora_B, np.float32)
    B, S, D_in = x.shape
    D_out = bias.shape[0]
    T_total = B * S
    T = T_total // N_CORES

    W_eff = host_fold_lora(W, bias, lora_A, lora_B)
    x2d = np.ascontiguousarray(x.reshape(T_total, D_in))

    key = (D_in, D_out, T)
    if key not in _nc_cache:
        _nc_cache[key] = build_bass(D_in, D_out, T)
    nc = _nc_cache[key]

    in_maps = _prep_core_inputs(x2d, W_eff, bias, D_in, D_out, T, N_CORES)
    res = run_bass_kernel_spmd(nc, in_maps, list(range(N_CORES)), trace=trace)

    out = np.empty((T_total, D_out), dtype=np.float32)
    for c in range(N_CORES):
        out[c * T:(c + 1) * T] = res.results[c]["out"].astype(np.float32).T
    out = out.reshape(B, S, D_out)
    if trace:
        return out, res
    return out


# revision 31
# speedup vs baseline: 1.0131x; 1.0068x over previous
"""Trainium2 Bass kernel for nn_MultiHeadMuonLoRALinear.

Math: out = x @ W^T + bias + sum_h alpha_h * x @ M_h^T, where
M_h = newtonschulz5(B_h @ A_h) and G_h = B_h @ A_h has rank hr=4.

Key algebraic identity: with G = B A (rank hr), every Newton-Schulz
iterate stays in the same row/column space, so X_k = B C_k A for an
hr x hr matrix C_k:
    C_0 = I / (||G||_F + eps),  ||G||_F^2 = tr((B^T B)(A A^T))
    C'  = a C + b (C P C^T) Q C + c (C P C^T Q)^2 C,  P = A A^T, Q = B^T B
Therefore M_h = B_h C_h A_h and the whole LoRA branch collapses to a
rank-16 update:  delta = sum_h alpha_h B_h C_h A_h,  out = x @ (W + delta)^T + bias.

The device kernel computes the single large GEMM (data-parallel over
tokens across 8 cores) with the rank-16 delta folded into W on the
host (0.2% of total FLOPs).

Device strategy (per core, T=1024 tokens, K=O=4096):
  - All-bf16 GEMM, f32 PSUM. The PE streams 1 moving column/cycle for
    every dtype >= bf16 (fp8 DoubleRow is the only 2x mode but fails
    the 2e-2 accuracy budget), so the per-core compute floor is
    32*32*2 matmuls x 512 columns = 437us. bf16 halves DMA vs fp32r
    and, critically, enables FWL (fast weight load), which fp32r's
    FP32-HIGH mode disables — LDWEIGHTS becomes fully hidden and the
    steady matmul issue rate drops from 233ns to 216ns.
  - Phase A k-sweeps the first A=4 o-tiles across the four double-bank
    PSUM groups, consuming x tiles in their DMA arrival order. The x
    tiles and phase-A weight chunks are spread across all three DMA
    queues (sync/scalar HWDGE ~100 GB/s each, gpsimd SWDGE ~210 GB/s)
    in a just-in-time order so the PE never starves while x streams.
  - Phase B runs the remaining o-tiles k-contiguous, tb-outer so each
    half's activation+store overlaps the other half's matmuls; W slabs
    are prefetched on the gpsimd queue, throttled by the pool depth.
  - A short PE warmup (dep-free matmuls) covers the fixed ~9us NEFF
    preamble + first-tile DMA window so the HAM clock gate is released
    before the first real matmul.
"""

import numpy as np
import ml_dtypes

import concourse.bass as bass
import concourse.bacc as bacc
import concourse.mybir as mybir
import concourse.tile as tile
from concourse.bass import ts
from concourse.bass_utils import run_bass_kernel_spmd

N_HEADS = 4
NS_STEPS = 5
NS_EPS = 1e-7
NS_A, NS_B, NS_C = 3.4445, -4.775, 2.0315

N_CORES = 8
P = 128

F32 = mybir.dt.float32
BF16 = mybir.dt.bfloat16

A_OT = 4       # phase-A o-tiles (PSUM: 4 groups x 2 banks = all 8 banks)
KC = 4         # k-tiles per phase-A weight chunk
WARMUP = 11


def host_fold_lora(W, bias, lora_A, lora_B):
    """Collapse the per-head Newton-Schulz into hr x hr space (float64)
    and return W_eff = W + sum_h alpha_h B_h C_h A_h (float32)."""
    r, D_in = lora_A.shape
    D_out = lora_B.shape[0]
    hr = r // N_HEADS
    Ah = lora_A.reshape(N_HEADS, hr, D_in).astype(np.float64)
    Bh = lora_B.reshape(D_out, N_HEADS, hr).transpose(1, 0, 2).astype(np.float64)

    AT = np.zeros((r, D_in))   # rows: alpha-weighted C_h A_h per head
    BT = np.zeros((D_out, r))  # cols: B_h per head
    for h in range(N_HEADS):
        A = Ah[h]
        B = Bh[h]
        Pm = A @ A.T
        Qm = B.T @ B
        fro = np.sqrt(np.trace(Qm @ Pm))
        C = np.eye(hr) / (fro + NS_EPS)
        for _ in range(NS_STEPS):
            D = C @ Pm @ C.T
            E = D @ Qm
            C = NS_A * C + NS_B * (E @ C) + NS_C * (E @ (E @ C))
        AT[h * hr:(h + 1) * hr] = fro * (C @ A)
        BT[:, h * hr:(h + 1) * hr] = B
    delta = BT @ AT
    return (W.astype(np.float64) + delta).astype(np.float32)


def build_bass(K, O, T):
    """Per-core SPMD program: outT[O, T] = (x W_eff^T + bias)^T for this
    core's token shard, all-bf16 GEMM with f32 PSUM accumulation.

    DRAM layouts (host-prepared):
      x:    [128, K//128, T] bf16   x_dev[k, kt, t] = x_shard[t, kt*128 + k]
      w:    [O//128, 128, K//128, 128] bf16  w_dev[ot, k, kt, o] = W_eff[ot*128+o, kt*128+k]
      wa:   [CH, 128, A, KC, 128] bf16  phase-A chunk groups:
            wa_dev[c, k, a, kc, o] = W_eff[a*128+o, (c*KC+kc)*128+k]
      bias: [128, O//128] f32       bias_dev[o, ot] = bias[ot*128 + o]
      out:  [O, T] bf16             outT
    """
    KT, OT = K // P, O // P
    A = A_OT
    CH = KT // KC
    TB = T // 512
    nc = bacc.Bacc()

    x_d = nc.declare_dram_parameter("x", [P, KT, T], BF16, isOutput=False)
    w_d = nc.declare_dram_parameter("w", [OT, P, KT, P], BF16, isOutput=False)
    wa_d = nc.declare_dram_parameter("wa", [CH, P, A, KC, P], BF16,
                                     isOutput=False)
    pkt_d = nc.declare_dram_parameter("pkt", [P, 2 * A * P + 512], BF16,
                                      isOutput=False)
    b_d = nc.declare_dram_parameter("bias", [P, OT], F32, isOutput=False)
    out_d = nc.declare_dram_parameter("out", [O, T], BF16, isOutput=True)

    with tile.TileContext(nc) as tc:
        with (
            tc.tile_pool(name="xpool", bufs=1) as xpool,
            tc.tile_pool(name="cpool", bufs=1) as cpool,
            tc.tile_pool(name="wapool", bufs=1) as wapool,
            tc.tile_pool(name="wbpool", bufs=6) as wbpool,
            tc.tile_pool(name="opool", bufs=6) as opool,
            tc.tile_pool(name="pspool", bufs=4, space="PSUM") as pspool,
        ):
            # x access: x_tb[kt][tb] -> [128, 512] AP for that k-tile half.
            x_tb = [None] * KT

            def load_x(kt, engine):
                xt = xpool.tile([P, T], BF16, tag=f"x{kt}", name=f"x{kt}")
                engine.dma_start(out=xt[:], in_=x_d[:, kt, :])
                x_tb[kt] = [xt[:, ts(tb, 512)] for tb in range(TB)]

            def load_x_halves(kt, engine):
                # Two half-tiles so the first 512 tokens land ~2us sooner;
                # used for the k-tiles that gate the phase-A start.
                aps = []
                for tb in range(TB):
                    xt = xpool.tile([P, 512], BF16, tag=f"x{kt}_{tb}",
                                    name=f"x{kt}_{tb}")
                    engine.dma_start(out=xt[:],
                                     in_=x_d[:, kt, ts(tb, 512)])
                    aps.append(xt[:])
                x_tb[kt] = aps

            def load_x_batch(k0, k1, engine):
                n = k1 - k0
                xt = xpool.tile([P, n, T], BF16, tag=f"xb{k0}",
                                name=f"xb{k0}")
                engine.dma_start(out=xt[:], in_=x_d[:, k0:k1, :])
                for kt in range(k0, k1):
                    x_tb[kt] = [xt[:, kt - k0, ts(tb, 512)]
                                for tb in range(TB)]

            # Phase-A weights: wlhsT(a, kt) -> [128, 128] stationary AP.
            wa_group = {}    # c -> tile [P, A, KC, P]
            # First packet: kt0+kt1 weights for all A o-tiles plus the
            # first 512-token half of x[kt0], as ONE dma on the fast
            # gpsimd queue so the phase-A start is deterministic (~12us)
            # instead of racing the contended HWDGE queues.
            pkt = [None]

            def load_pkt(engine):
                wt = wapool.tile([P, 2 * A * P + 512], BF16, tag="pkt",
                                 name="pkt")
                engine.dma_start(out=wt[:], in_=pkt_d[:])
                pkt[0] = wt

            def load_wa_group(c, engine):
                # kt0/kt1 weights already ride in the first packet, so
                # group 0 loads only its last KC-2 k-tiles — keeps the
                # gpsimd pkt->c0->c1 critical chain short.
                k0 = 2 if c == 0 else 0
                wt = wapool.tile([P, A, KC - k0, P], BF16, tag=f"wag{c}",
                                 name=f"wag{c}")
                engine.dma_start(out=wt[:], in_=wa_d[c, :, :, k0:, :])
                wa_group[c] = wt

            def wa_lhsT(a, kt):
                if kt < 2 and pkt[0] is not None:
                    return pkt[0][:, (kt * A + a) * P:(kt * A + a + 1) * P]
                c = kt // KC
                kc = kt % KC - (2 if c == 0 else 0)
                return wa_group[c][:, a, kc, :]

            wb = {}          # ot -> slab tile [P, KT, P]

            def load_wb(ot, engine):
                wt = wbpool.tile([P, KT, P], BF16, tag="wb", name=f"w{ot}")
                engine.dma_start(out=wt[:], in_=w_d[ot])
                wb[ot] = wt

            bias_sb = cpool.tile([P, OT], F32)

            # ---- queue programs (emission order == per-engine queue order)
            # Just-in-time supply: phase A consumes (x[kt], wa chunk) pairs
            # at ~1.73us per k-tile from t~14us; each item below lands
            # (at ~100 GB/s HWDGE / ~210 GB/s SWDGE) ahead of its deadline.
            # sync HWDGE (~55 GB/s while contended early, ~100 after):
            # x0's first half rides in the gpsimd first packet; load only
            # its second half here.
            x0h1 = xpool.tile([P, 512], BF16, tag="x0_1", name="x0_1")
            nc.sync.dma_start(out=x0h1[:], in_=x_d[:, 0, ts(1, 512)])
            for kt in (2, 3, 4, 6, 12, 14):
                load_x(kt, nc.sync)
            load_x_batch(16, 20, nc.sync)
            load_x_batch(24, 28, nc.sync)
            # scalar HWDGE:
            load_x_halves(1, nc.scalar)
            for kt in (5, 7, 13, 15):
                load_x(kt, nc.scalar)
            load_x_batch(20, 24, nc.scalar)
            load_x_batch(28, 32, nc.scalar)
            nc.scalar.dma_start(out=bias_sb[:], in_=b_d[:])
            # gpsimd SWDGE (~190 GB/s, ~1.3us fixed/item): all phase-A
            # weight chunk groups plus the mid x batch, in deadline order.
            load_pkt(nc.gpsimd)
            load_wa_group(0, nc.gpsimd)
            load_wa_group(1, nc.gpsimd)
            load_x_batch(8, 12, nc.gpsimd)
            load_wa_group(2, nc.gpsimd)
            load_wa_group(3, nc.gpsimd)
            load_wa_group(4, nc.gpsimd)
            load_wa_group(5, nc.gpsimd)
            load_wa_group(6, nc.gpsimd)
            load_wa_group(7, nc.gpsimd)
            load_wb(4, nc.gpsimd)
            load_wb(5, nc.gpsimd)
            x_tb[0] = [pkt[0][:, 2 * A * P:2 * A * P + 512], x0h1[:]]

            # PE warmup across the preamble + first-tile DMA window.
            wu_src = cpool.tile([P, 512], BF16, name="wu_src")
            nc.vector.memset(wu_src[:], 0.0)
            wu_ps = pspool.tile([P, T], F32, tag="ps", name="wu_ps")
            for i in range(WARMUP):
                nc.tensor.matmul(
                    wu_ps[:, :512], lhsT=wu_src[:, :P], rhs=wu_src[:],
                    start=(i == 0), stop=(i == WARMUP - 1),
                )

            # Phase A: k-outer sweep over the first A o-tiles in parallel
            # PSUM groups. o-tile order 2,3,0,1 at each k so the first
            # matmuls wait on the earliest-arriving weight chunks.
            ps_a = [pspool.tile([P, T], F32, tag="ps", name=f"psA{a}")
                    for a in range(A)]
            for kt in range(KT):
                for a in range(A):
                    for tb in range(TB):
                        nc.tensor.matmul(
                            ps_a[a][:, ts(tb, 512)],
                            lhsT=wa_lhsT(a, kt),
                            rhs=x_tb[kt][tb],
                            start=(kt == 0),
                            stop=(kt == KT - 1),
                        )

            def emit_part(ot, ps, j, width):
                out_sb = opool.tile([P, width], BF16)
                nc.scalar.activation(
                    out_sb[:],
                    ps[:, ts(j, width)],
                    mybir.ActivationFunctionType.Identity,
                    bias=bias_sb[:, ot:ot + 1],
                )
                eng = nc.sync if ot % 2 == 0 else nc.scalar
                eng.dma_start(out=out_d[ts(ot, P), ts(j, width)],
                              in_=out_sb[:])

            def emit_half(ot, ps, tb):
                emit_part(ot, ps, tb, 512)

            for a in range(A):
                for tb in range(TB):
                    emit_half(a, ps_a[a], tb)

            # Phase B: k-contiguous, tb-outer so each half's ACT+store
            # overlaps the other half's matmuls; W slabs prefetched on the
            # gpsimd SWDGE queue (pool-depth throttled).
            for ot in range(A, OT):
                if ot not in wb:
                    load_wb(ot, nc.gpsimd)
                wt = wb[ot]
                # taper the final o-tile so only a 256-wide ACT+store
                # trails the last matmul; each group gets its own PSUM
                # tile so a group's matmuls don't false-depend on the
                # previous group's ACT (tile-granular tracking)
                spans = ([(0, 512), (512, 256), (768, 256)]
                         if ot == OT - 1 else [(0, 512), (512, 512)])
                for off, width in spans:
                    ps = pspool.tile([P, width], F32, tag="ps",
                                     name=f"ps{ot}_{off}")
                    for kt in range(KT):
                        nc.tensor.matmul(
                            ps[:],
                            lhsT=wt[:, kt, :],
                            rhs=x_tb[kt][off // 512][:, off % 512:off % 512 + width]
                                if width != 512 else x_tb[kt][off // 512],
                            start=(kt == 0),
                            stop=(kt == KT - 1),
                        )
                    out_sb = opool.tile([P, width], BF16)
                    nc.scalar.activation(
                        out_sb[:],
                        ps[:],
                        mybir.ActivationFunctionType.Identity,
                        bias=bias_sb[:, ot:ot + 1],
                    )
                    eng = nc.sync if ot % 2 == 0 else nc.scalar
                    eng.dma_start(out=out_d[ts(ot, P), off:off + width],
                                  in_=out_sb[:])

    nc.compile()
    return nc


def _prep_core_inputs(x2d, W_eff, bias, K, O, T, n_cores):
    """Host-side layout prep: shard tokens, make partition-major layouts."""
    KT, OT = K // P, O // P
    A, CH = A_OT, KT // KC
    w_bf = W_eff.astype(ml_dtypes.bfloat16)
    w_dev = np.ascontiguousarray(
        w_bf.reshape(OT, P, KT, P).transpose(0, 3, 2, 1)
    )  # [ot, k, kt, o]
    # [c, k, a, kc, o] from W_eff[a*128+o, (c*KC+kc)*128+k]
    wa_dev = np.ascontiguousarray(
        w_bf[:A * P].reshape(A, P, CH, KC, P).transpose(2, 4, 0, 3, 1)
    )
    bias_dev = np.ascontiguousarray(bias.reshape(OT, P).T)  # [o(part), ot]
    # first-packet weights: kt0+kt1 blocks for the A phase-A o-tiles,
    # laid out [k, (kt*A + a)*P + o] = W_eff[a*128+o, kt*128+k]
    wpkt = np.ascontiguousarray(
        w_bf[:A * P, :2 * P].reshape(A, P, 2, P).transpose(3, 2, 0, 1)
        .reshape(P, 2 * A * P)
    )
    in_maps = []
    for c in range(n_cores):
        xs = x2d[c * T:(c + 1) * T]  # [T, K]
        x_dev = np.ascontiguousarray(
            xs.astype(ml_dtypes.bfloat16).reshape(T, KT, P).transpose(2, 1, 0)
        )  # [k, kt, t]
        pkt_dev = np.ascontiguousarray(
            np.concatenate([wpkt, x_dev[:, 0, :512]], axis=1)
        )
        in_maps.append({"x": x_dev, "w": w_dev, "wa": wa_dev,
                        "bias": bias_dev, "pkt": pkt_dev})
    return in_maps


def kernel(x, W, bias, lora_A, lora_B, trace=False, _nc_cache={}):
    x = np.asarray(x, np.float32)
    W = np.asarray(W, np.float32)
    bias = np.asarray(bias, np.float32)
    lora_A = np.asarray(lora_A, np.float32)
    lora_B = np.asarray(lora_B, np.float32)
    B, S, D_in = x.shape
    D_out = bias.shape[0]
    T_total = B * S
    T = T_total // N_CORES

    W_eff = host_fold_lora(W, bias, lora_A, lora_B)
    x2d = np.ascontiguousarray(x.reshape(T_total, D_in))

    key = (D_in, D_out, T)
    if key not in _nc_cache:
        _nc_cache[key] = build_bass(D_in, D_out, T)
    nc = _nc_cache[key]

    in_maps = _prep_core_inputs(x2d, W_eff, bias, D_in, D_out, T, N_CORES)
    res = run_bass_kernel_spmd(nc, in_maps, list(range(N_CORES)), trace=trace)

    out = np.empty((T_total, D_out), dtype=np.float32)
    for c in range(N_CORES):
        out[c * T:(c + 1) * T] = res.results[c]["out"].astype(np.float32).T
    out = out.reshape(B, S, D_out)
    if trace:
        return out, res
    return out
